# revision 12
# baseline (speedup 1.0000x reference)
"""GAT-style attention kernel for Trainium2, data-parallel over batch on 8 cores.

Math (see derivation in comments below): the reference computes
    e[i,j]  = lr_row[i] + lr_col[j]            (rank-1 score structure)
    atten   = softmax_j(where(mask>0, e, -1e9))
    out     = atten @ (x @ Wx.T + bx)
Because lr_row[i] is constant along the softmax axis j, it cancels:
    atten[i,j] = mask[i,j] * w[j] / sum_j mask[i,j] * w[j],
    w[j] = exp(lr_col[j] - max_j lr_col[j])
and since attention rows sum to 1, the bias bx passes through unchanged:
    out = (M @ (w * xv0)) / (M @ w) + bx,   xv0 = x @ Wx.T
So the whole kernel is one [N,N] x [N,129] matmul per batch, normalized
row-wise, with tiny setup.  Memory-bound on the int32 mask read (16MB/core).

Per core (batch b):
  - mask strips [128, N] are DMA-loaded with SWDGE int32->bf16 cast
  - xbar DMA-transpose produces maskT chunks [j_in, j_blk, i] in SBUF
  - PE accumulates psum[i, 132] over 16 j-chunks: lhsT=maskT chunk (bf16),
    rhs=U chunk [128, 132] where U[:, :128] = w*xv0, U[:, 128] = w
  - normalize by column 128, add bx, store f32
"""

import os
import sys

import numpy as np

for _p in ("/opt/trn_rl_repo",):
    if _p not in sys.path and os.path.isdir(_p):
        sys.path.append(_p)

import concourse.bacc as bacc
import concourse.bass as bass
import concourse.bass_isa as bass_isa
import concourse.tile as tile
from concourse import mybir
from concourse.bass_utils import run_bass_kernel_spmd

B, N, DIN, DOUT, DA = 8, 2048, 128, 128, 2
NEG_SLOPE = 0.2
P = 128
UC = 132  # U free width: 128 numerator cols + 1 denom col + 3 pad

F32 = mybir.dt.float32
BF16 = mybir.dt.bfloat16
I32 = mybir.dt.int32


def build(n=N, mask_bufs=3, use_3d_xbar=True, variant="hwdge_split", cast_cols_dve=1536,
          xpose_queues=("sync",), load_engine="alt"):
    """Build the single-core program (all 8 cores run it SPMD).

    variant:
      "swdge_cast":  SWDGE cast-DMA loads + xbar transposes on sync (v1; slow)
      "hwdge_split": plain int32 HWDGE loads, DVE+GpSimd cast, xbar transposes
                     split across sync+scalar queues
    """
    nt = n // P
    nc = bacc.Bacc(
        "TRN2",
        target_bir_lowering=False,
        debug=False,
        enable_asserts=False,
        num_devices=1,
    )
    x_d = nc.dram_tensor("x", [n, DIN], F32, kind="ExternalInput").ap()
    m_d = nc.dram_tensor("mask", [n, n], I32, kind="ExternalInput").ap()
    # wcomb = [Wx.T | Wc.T]  (precomputed on host; tiny params)
    wcomb_d = nc.dram_tensor("wcomb", [DIN, DOUT + DA], F32, kind="ExternalInput").ap()
    a2_d = nc.dram_tensor("a2", [1, DA], F32, kind="ExternalInput").ap()
    bx_d = nc.dram_tensor("bx", [1, DOUT], F32, kind="ExternalInput").ap()
    ident_d = nc.dram_tensor("ident", [P, P], F32, kind="ExternalInput").ap()
    out_d = nc.dram_tensor("out", [n, DOUT], F32, kind="ExternalOutput").ap()

    from contextlib import ExitStack

    with tile.TileContext(nc) as tc, ExitStack() as ctx:
        consts = ctx.enter_context(tc.tile_pool(name="consts", bufs=1))
        small = ctx.enter_context(tc.tile_pool(name="small", bufs=2))
        mpool = ctx.enter_context(tc.tile_pool(name="mpool", bufs=mask_bufs))
        cpool = ctx.enter_context(tc.tile_pool(name="cpool", bufs=mask_bufs))
        tpool = ctx.enter_context(tc.tile_pool(name="tpool", bufs=mask_bufs))
        opool = ctx.enter_context(tc.tile_pool(name="opool", bufs=3))
        ps_small = ctx.enter_context(tc.tile_pool(name="ps_small", bufs=2, space="PSUM"))
        ps_acc = ctx.enter_context(tc.tile_pool(name="ps_acc", bufs=2, space="PSUM"))

        # ---- constants ----
        ident = consts.tile([P, P], F32)
        nc.sync.dma_start(ident[:], ident_d)
        wcomb = consts.tile([DIN, DOUT + DA], F32)
        nc.sync.dma_start(wcomb[:], wcomb_d)
        a2s = consts.tile([1, DA], F32)
        nc.sync.dma_start(a2s[:], a2_d)
        bxs = consts.tile([1, DOUT], F32)
        nc.sync.dma_start(bxs[:], bx_d)
        a2b = consts.tile([P, DA], F32)
        nc.gpsimd.partition_broadcast(a2b[:], a2s[:])
        bxb = consts.tile([P, DOUT], F32)
        nc.gpsimd.partition_broadcast(bxb[:], bxs[:])

        # ---- x -> xT via PE transpose ----
        x_nat = consts.tile([P, nt, DIN], F32)
        nc.sync.dma_start(x_nat[:], x_d.rearrange("(t p) d -> p t d", p=P))
        xT = consts.tile([P, n], F32)
        for t in range(nt):
            ps = ps_small.tile([P, P], F32)
            nc.tensor.transpose(ps[:], x_nat[:, t], ident[:])
            nc.scalar.copy(xT[:, t * P : (t + 1) * P], ps[:])

        # ---- projections: psum[j, 130] = xT_chunk.T @ [WxT | WcT] ----
        xv_sb = consts.tile([P, nt, DOUT], F32)
        lrc = consts.tile([P, nt], F32)
        for t in range(nt):
            pxv = ps_small.tile([P, DOUT + DA], F32)
            nc.tensor.matmul(
                pxv[:], xT[:, t * P : (t + 1) * P], wcomb[:], start=True, stop=True
            )
            nc.scalar.copy(xv_sb[:, t], pxv[:, 0:DOUT])
            # LeakyReLU(col) = max(col, 0.2*col); col = pxv[:, 128:130]
            c02 = small.tile([P, DA], F32)
            nc.vector.tensor_scalar_mul(c02[:], pxv[:, DOUT : DOUT + DA], NEG_SLOPE)
            clr = small.tile([P, DA], F32)
            nc.vector.tensor_max(clr[:], pxv[:, DOUT : DOUT + DA], c02[:])
            # lr_col_t = clr[:,0]*a2[0] + clr[:,1]*a2[1]
            p0 = small.tile([P, 1], F32)
            nc.vector.tensor_scalar(
                p0[:], clr[:, 0:1], a2b[:, 0:1], None, mybir.AluOpType.mult
            )
            p1 = small.tile([P, 1], F32)
            nc.vector.tensor_scalar(
                p1[:], clr[:, 1:2], a2b[:, 1:2], None, mybir.AluOpType.mult
            )
            nc.vector.tensor_add(lrc[:, t : t + 1], p0[:], p1[:])

        # ---- global max over all j, w = exp(lrc - max) ----
        mx = small.tile([P, 1], F32)
        nc.vector.tensor_reduce(
            mx[:], lrc[:], axis=mybir.AxisListType.X, op=mybir.AluOpType.max
        )
        mxr = small.tile([P, 1], F32)
        nc.gpsimd.partition_all_reduce(
            mxr[:], mx[:], channels=P, reduce_op=bass_isa.ReduceOp.max
        )
        negmx = small.tile([P, 1], F32)
        nc.vector.tensor_scalar_mul(negmx[:], mxr[:], -1.0)
        w_all = consts.tile([P, nt], F32)
        nc.scalar.activation(
            w_all[:], lrc[:], mybir.ActivationFunctionType.Exp, bias=negmx[:]
        )

        # ---- U chunks: [P, UC] bf16 ----
        U = consts.tile([P, nt, UC], BF16)
        for t in range(nt):
            nc.scalar.activation(
                U[:, t, 0:DOUT],
                xv_sb[:, t],
                mybir.ActivationFunctionType.Copy,
                scale=w_all[:, t : t + 1],
            )
            nc.vector.tensor_copy(U[:, t, DOUT : DOUT + 1], w_all[:, t : t + 1])
            nc.vector.memset(U[:, t, DOUT + 1 : UC], 0)

        raw = consts.tile([P, nt, UC], F32)

        # ---- main loop over output row strips ----
        for ti in range(nt):
            if variant == "swdge_cast":
                mbf = mpool.tile([P, n], BF16)
                nc.gpsimd.dma_start(mbf[:], m_d[ti * P : (ti + 1) * P, :])
                mT = tpool.tile([P, nt, P], BF16)
                if use_3d_xbar:
                    nc.sync.dma_start(mT[:], mbf[:], transpose=True)
                else:
                    for tj in range(nt):
                        nc.sync.dma_start(
                            mT[:, tj], mbf[:, tj * P : (tj + 1) * P], transpose=True
                        )
            else:
                mi32 = mpool.tile([P, n], I32)
                if load_engine == "gpsimd":
                    ldq = nc.gpsimd
                elif load_engine == "alt":
                    ldq = nc.sync if ti % 2 == 0 else nc.scalar
                else:
                    ldq = getattr(nc, load_engine)
                ldq.dma_start(mi32[:], m_d[ti * P : (ti + 1) * P, :])
                mbf = cpool.tile([P, n], BF16)
                cc = max(P, min(n - P, cast_cols_dve * n // N))
                nc.vector.tensor_copy(mbf[:, 0:cc], mi32[:, 0:cc])
                nc.gpsimd.tensor_copy(mbf[:, cc:n], mi32[:, cc:n])
                mT = tpool.tile([P, nt, P], BF16)
                qs = [getattr(nc, q) for q in xpose_queues]
                nq = len(qs)
                h = nt // nq
                for qi, q in enumerate(qs):
                    q.dma_start(
                        mT[:, qi * h : (qi + 1) * h],
                        mbf[:, qi * h * P : (qi + 1) * h * P],
                        transpose=True,
                    )
            pacc = ps_acc.tile([P, UC], F32)
            for tj in range(nt):
                nc.tensor.matmul(
                    pacc[:],
                    mT[:, tj],
                    U[:, tj],
                    start=(tj == 0),
                    stop=(tj == nt - 1),
                )
            # phase A: just evacuate raw PSUM on ACT; normalize later so
            # DVE stays dedicated to mask casts during streaming
            nc.scalar.copy(raw[:, ti], pacc[:])

        # ---- phase B: normalize + bias + store ----
        for ti in range(nt):
            rec = small.tile([P, 1], F32)
            nc.vector.reciprocal(rec[:], raw[:, ti, DOUT : DOUT + 1])
            o1 = opool.tile([P, DOUT], F32)
            nc.scalar.activation(
                o1[:], raw[:, ti, 0:DOUT], mybir.ActivationFunctionType.Copy,
                scale=rec[:],
            )
            o2 = opool.tile([P, DOUT], F32)
            nc.vector.tensor_add(o2[:], o1[:], bxb[:])
            nc.scalar.dma_start(out_d[ti * P : (ti + 1) * P, :], o2[:])

    nc.compile()
    return nc


def host_inputs(x, mask, Wc, Wcat, Wx, bx, b):
    """Per-core input map for batch b (weights replicated, host-prepped)."""
    return {
        "x": np.ascontiguousarray(x[b], dtype=np.float32),
        "mask": np.ascontiguousarray(mask[b], dtype=np.int32),
        "wcomb": np.ascontiguousarray(
            np.concatenate([Wx.T, Wc.T], axis=1), dtype=np.float32
        ),
        "a2": np.ascontiguousarray(Wcat[DA:].reshape(1, DA), dtype=np.float32),
        "bx": np.ascontiguousarray(bx.reshape(1, DOUT), dtype=np.float32),
        "ident": np.eye(P, dtype=np.float32),
    }


_cached = {}


def _get_nc():
    if "nc" not in _cached:
        _cached["nc"] = build()
    return _cached["nc"]


def _install_ntff_shim():
    """The agent image's antenv lacks axon_hooks; synthesize it so
    run_bass_kernel_spmd(trace=True) can reach the .so's NTFF profiler."""
    import types

    try:
        import antenv.axon_hooks  # noqa: F401

        return True
    except ImportError:
        pass
    try:
        import antenv
        from trn_agent_boot.trn_boot import _ntff_profile_via_ctypes

        hook = _ntff_profile_via_ctypes("/opt/axon/libaxon_pjrt.so")
        mod = types.ModuleType("antenv.axon_hooks")
        _state = {"hook": hook}
        mod.set_axon_ntff_profile_hook = lambda h: _state.__setitem__("hook", h)
        mod.get_axon_ntff_profile_hook = lambda: _state["hook"]
        sys.modules["antenv.axon_hooks"] = mod
        antenv.axon_hooks = mod
        return hook is not None
    except Exception as e:
        print(f"ntff shim failed: {e}", file=sys.stderr)
        return False


def kernel(x, mask, Wr, Wc, Wcat, Wx, bx, _trace=False, **_unused):
    x = np.asarray(x)
    mask = np.asarray(mask)
    Wc = np.asarray(Wc)
    Wcat = np.asarray(Wcat)
    Wx = np.asarray(Wx)
    bx = np.asarray(bx)
    nc = _get_nc()
    if _trace:
        _trace = _install_ntff_shim()
    in_maps = [host_inputs(x, mask, Wc, Wcat, Wx, bx, b) for b in range(B)]
    res = run_bass_kernel_spmd(nc, in_maps, core_ids=list(range(B)), trace=_trace)
    out = np.stack([res.results[c]["out"] for c in range(B)]).astype(np.float32)
    if _trace:
        kernel.last_results = res
    return out


# revision 13
# speedup vs baseline: 1.1152x; 1.1152x over previous
"""GAT-style attention kernel for Trainium2, data-parallel over batch on 8 cores.

Math (see derivation in comments below): the reference computes
    e[i,j]  = lr_row[i] + lr_col[j]            (rank-1 score structure)
    atten   = softmax_j(where(mask>0, e, -1e9))
    out     = atten @ (x @ Wx.T + bx)
Because lr_row[i] is constant along the softmax axis j, it cancels:
    atten[i,j] = mask[i,j] * w[j] / sum_j mask[i,j] * w[j],
    w[j] = exp(lr_col[j] - max_j lr_col[j])
and since attention rows sum to 1, the bias bx passes through unchanged:
    out = (M @ (w * xv0)) / (M @ w) + bx,   xv0 = x @ Wx.T
So the whole kernel is one [N,N] x [N,129] matmul per batch, normalized
row-wise, with tiny setup.  Memory-bound on the int32 mask read (16MB/core).

Per core (batch b):
  - mask strips [128, N] are DMA-loaded with SWDGE int32->bf16 cast
  - xbar DMA-transpose produces maskT chunks [j_in, j_blk, i] in SBUF
  - PE accumulates psum[i, 132] over 16 j-chunks: lhsT=maskT chunk (bf16),
    rhs=U chunk [128, 132] where U[:, :128] = w*xv0, U[:, 128] = w
  - normalize by column 128, add bx, store f32
"""

import os
import sys

import numpy as np

for _p in ("/opt/trn_rl_repo",):
    if _p not in sys.path and os.path.isdir(_p):
        sys.path.append(_p)

import concourse.bacc as bacc
import concourse.bass as bass
import concourse.bass_isa as bass_isa
import concourse.tile as tile
from concourse import mybir
from concourse.bass_utils import run_bass_kernel_spmd

B, N, DIN, DOUT, DA = 8, 2048, 128, 128, 2
NEG_SLOPE = 0.2
P = 128
UC = 132  # U free width: 128 numerator cols + 1 denom col + 3 pad

F32 = mybir.dt.float32
BF16 = mybir.dt.bfloat16
I32 = mybir.dt.int32


def build(n=N, mask_bufs=3, use_3d_xbar=True, variant="hwdge_split", cast_cols_dve=1536,
          xpose_queues=("sync",), load_engine="alt"):
    """Build the single-core program (all 8 cores run it SPMD).

    variant:
      "swdge_cast":  SWDGE cast-DMA loads + xbar transposes on sync (v1; slow)
      "hwdge_split": plain int32 HWDGE loads, DVE+GpSimd cast, xbar transposes
                     split across sync+scalar queues
    """
    nt = n // P
    nc = bacc.Bacc(
        "TRN2",
        target_bir_lowering=False,
        debug=False,
        enable_asserts=False,
        num_devices=1,
    )
    x_d = nc.dram_tensor("x", [n, DIN], F32, kind="ExternalInput").ap()
    m_d = nc.dram_tensor("mask", [n, n], I32, kind="ExternalInput").ap()
    # wcomb = [Wx.T | Wc.T]  (precomputed on host; tiny params)
    wcomb_d = nc.dram_tensor("wcomb", [DIN, DOUT + DA], F32, kind="ExternalInput").ap()
    a2_d = nc.dram_tensor("a2", [1, DA], F32, kind="ExternalInput").ap()
    bx_d = nc.dram_tensor("bx", [1, DOUT], F32, kind="ExternalInput").ap()
    ident_d = nc.dram_tensor("ident", [P, P], F32, kind="ExternalInput").ap()
    out_d = nc.dram_tensor("out", [n, DOUT], F32, kind="ExternalOutput").ap()

    from contextlib import ExitStack

    with tile.TileContext(nc) as tc, ExitStack() as ctx:
        consts = ctx.enter_context(tc.tile_pool(name="consts", bufs=1))
        small = ctx.enter_context(tc.tile_pool(name="small", bufs=2))
        mpool = ctx.enter_context(tc.tile_pool(name="mpool", bufs=mask_bufs + 1))
        cpool = ctx.enter_context(tc.tile_pool(name="cpool", bufs=mask_bufs))
        tpool = ctx.enter_context(tc.tile_pool(name="tpool", bufs=mask_bufs))
        opool = ctx.enter_context(tc.tile_pool(name="opool", bufs=3))
        ps_small = ctx.enter_context(tc.tile_pool(name="ps_small", bufs=2, space="PSUM"))
        ps_acc = ctx.enter_context(tc.tile_pool(name="ps_acc", bufs=4, space="PSUM"))

        # ---- constants ----
        ident = consts.tile([P, P], F32)
        nc.sync.dma_start(ident[:], ident_d)
        wcomb = consts.tile([DIN, DOUT + DA], F32)
        nc.sync.dma_start(wcomb[:], wcomb_d)
        a2s = consts.tile([1, DA], F32)
        nc.sync.dma_start(a2s[:], a2_d)
        bxs = consts.tile([1, DOUT], F32)
        nc.sync.dma_start(bxs[:], bx_d)
        a2b = consts.tile([P, DA], F32)
        nc.gpsimd.partition_broadcast(a2b[:], a2s[:])
        bxb = consts.tile([P, DOUT], F32)
        nc.gpsimd.partition_broadcast(bxb[:], bxs[:])

        # ---- x -> xT via PE transpose ----
        x_nat = consts.tile([P, nt, DIN], F32)
        nc.sync.dma_start(x_nat[:], x_d.rearrange("(t p) d -> p t d", p=P))
        xT = consts.tile([P, n], F32)
        for t in range(nt):
            ps = ps_small.tile([P, P], F32)
            nc.tensor.transpose(ps[:], x_nat[:, t], ident[:])
            nc.scalar.copy(xT[:, t * P : (t + 1) * P], ps[:])

        # ---- projections: psum[j, 130] = xT_chunk.T @ [WxT | WcT] ----
        xv_sb = consts.tile([P, nt, DOUT], F32)
        lrc = consts.tile([P, nt], F32)
        for t in range(nt):
            pxv = ps_small.tile([P, DOUT + DA], F32)
            nc.tensor.matmul(
                pxv[:], xT[:, t * P : (t + 1) * P], wcomb[:], start=True, stop=True
            )
            nc.scalar.copy(xv_sb[:, t], pxv[:, 0:DOUT])
            # LeakyReLU(col) = max(col, 0.2*col); col = pxv[:, 128:130]
            c02 = small.tile([P, DA], F32)
            nc.vector.tensor_scalar_mul(c02[:], pxv[:, DOUT : DOUT + DA], NEG_SLOPE)
            clr = small.tile([P, DA], F32)
            nc.vector.tensor_max(clr[:], pxv[:, DOUT : DOUT + DA], c02[:])
            # lr_col_t = clr[:,0]*a2[0] + clr[:,1]*a2[1]
            p0 = small.tile([P, 1], F32)
            nc.vector.tensor_scalar(
                p0[:], clr[:, 0:1], a2b[:, 0:1], None, mybir.AluOpType.mult
            )
            p1 = small.tile([P, 1], F32)
            nc.vector.tensor_scalar(
                p1[:], clr[:, 1:2], a2b[:, 1:2], None, mybir.AluOpType.mult
            )
            nc.vector.tensor_add(lrc[:, t : t + 1], p0[:], p1[:])

        # ---- global max over all j, w = exp(lrc - max) ----
        mx = small.tile([P, 1], F32)
        nc.vector.tensor_reduce(
            mx[:], lrc[:], axis=mybir.AxisListType.X, op=mybir.AluOpType.max
        )
        mxr = small.tile([P, 1], F32)
        nc.gpsimd.partition_all_reduce(
            mxr[:], mx[:], channels=P, reduce_op=bass_isa.ReduceOp.max
        )
        negmx = small.tile([P, 1], F32)
        nc.vector.tensor_scalar_mul(negmx[:], mxr[:], -1.0)
        w_all = consts.tile([P, nt], F32)
        nc.scalar.activation(
            w_all[:], lrc[:], mybir.ActivationFunctionType.Exp, bias=negmx[:]
        )

        # ---- U chunks: [P, UC] bf16 ----
        U = consts.tile([P, nt, UC], BF16)
        for t in range(nt):
            nc.scalar.activation(
                U[:, t, 0:DOUT],
                xv_sb[:, t],
                mybir.ActivationFunctionType.Copy,
                scale=w_all[:, t : t + 1],
            )
            nc.vector.tensor_copy(U[:, t, DOUT : DOUT + 1], w_all[:, t : t + 1])
            nc.vector.memset(U[:, t, DOUT + 1 : UC], 0)

        raw = consts.tile([P, nt, UC], F32)

        # ---- main loop over output row strips ----
        paccs = []
        for ti in range(nt):
            if variant == "swdge_cast":
                mbf = mpool.tile([P, n], BF16)
                nc.gpsimd.dma_start(mbf[:], m_d[ti * P : (ti + 1) * P, :])
                mT = tpool.tile([P, nt, P], BF16)
                if use_3d_xbar:
                    nc.sync.dma_start(mT[:], mbf[:], transpose=True)
                else:
                    for tj in range(nt):
                        nc.sync.dma_start(
                            mT[:, tj], mbf[:, tj * P : (tj + 1) * P], transpose=True
                        )
            else:
                mi32 = mpool.tile([P, n], I32)
                # sync (SP) queue is load-only: its waits never gate compute
                nc.sync.dma_start(mi32[:], m_d[ti * P : (ti + 1) * P, :])
                mbf = cpool.tile([P, n], BF16)
                cc = max(P, min(n - P, cast_cols_dve * n // N))
                nc.vector.tensor_copy(mbf[:, 0:cc], mi32[:, 0:cc])
                nc.gpsimd.tensor_copy(mbf[:, cc:n], mi32[:, cc:n])
                mT = tpool.tile([P, nt, P], BF16)
                # scalar (ACT) queue is transpose-only during the main loop
                nc.scalar.dma_start(mT[:], mbf[:], transpose=True)
            pacc = ps_acc.tile([P, UC], F32)
            paccs.append(pacc)
            for tj in range(nt):
                nc.tensor.matmul(
                    pacc[:],
                    mT[:, tj],
                    U[:, tj],
                    start=(tj == 0),
                    stop=(tj == nt - 1),
                )
            # evacuate PSUM on DVE with a 2-strip skew: by the time the copy
            # appears in DVE's program, the MMs it waits on are long done
            if ti >= 2:
                nc.vector.tensor_copy(raw[:, ti - 2], paccs[ti - 2][:])
        for ti in (nt - 2, nt - 1):
            nc.vector.tensor_copy(raw[:, ti], paccs[ti][:])

        # ---- phase B: normalize + bias + store ----
        for ti in range(nt):
            rec = small.tile([P, 1], F32)
            nc.vector.reciprocal(rec[:], raw[:, ti, DOUT : DOUT + 1])
            o1 = opool.tile([P, DOUT], F32)
            nc.scalar.activation(
                o1[:], raw[:, ti, 0:DOUT], mybir.ActivationFunctionType.Copy,
                scale=rec[:],
            )
            o2 = opool.tile([P, DOUT], F32)
            nc.vector.tensor_add(o2[:], o1[:], bxb[:])
            nc.scalar.dma_start(out_d[ti * P : (ti + 1) * P, :], o2[:])

    nc.compile()
    return nc


def host_inputs(x, mask, Wc, Wcat, Wx, bx, b):
    """Per-core input map for batch b (weights replicated, host-prepped)."""
    return {
        "x": np.ascontiguousarray(x[b], dtype=np.float32),
        "mask": np.ascontiguousarray(mask[b], dtype=np.int32),
        "wcomb": np.ascontiguousarray(
            np.concatenate([Wx.T, Wc.T], axis=1), dtype=np.float32
        ),
        "a2": np.ascontiguousarray(Wcat[DA:].reshape(1, DA), dtype=np.float32),
        "bx": np.ascontiguousarray(bx.reshape(1, DOUT), dtype=np.float32),
        "ident": np.eye(P, dtype=np.float32),
    }


_cached = {}


def _get_nc():
    if "nc" not in _cached:
        _cached["nc"] = build()
    return _cached["nc"]


def _install_ntff_shim():
    """The agent image's antenv lacks axon_hooks; synthesize it so
    run_bass_kernel_spmd(trace=True) can reach the .so's NTFF profiler."""
    import types

    try:
        import antenv.axon_hooks  # noqa: F401

        return True
    except ImportError:
        pass
    try:
        import antenv
        from trn_agent_boot.trn_boot import _ntff_profile_via_ctypes

        hook = _ntff_profile_via_ctypes("/opt/axon/libaxon_pjrt.so")
        mod = types.ModuleType("antenv.axon_hooks")
        _state = {"hook": hook}
        mod.set_axon_ntff_profile_hook = lambda h: _state.__setitem__("hook", h)
        mod.get_axon_ntff_profile_hook = lambda: _state["hook"]
        sys.modules["antenv.axon_hooks"] = mod
        antenv.axon_hooks = mod
        return hook is not None
    except Exception as e:
        print(f"ntff shim failed: {e}", file=sys.stderr)
        return False


def kernel(x, mask, Wr, Wc, Wcat, Wx, bx, _trace=False, **_unused):
    x = np.asarray(x)
    mask = np.asarray(mask)
    Wc = np.asarray(Wc)
    Wcat = np.asarray(Wcat)
    Wx = np.asarray(Wx)
    bx = np.asarray(bx)
    nc = _get_nc()
    if _trace:
        _trace = _install_ntff_shim()
    in_maps = [host_inputs(x, mask, Wc, Wcat, Wx, bx, b) for b in range(B)]
    res = run_bass_kernel_spmd(nc, in_maps, core_ids=list(range(B)), trace=_trace)
    out = np.stack([res.results[c]["out"] for c in range(B)]).astype(np.float32)
    if _trace:
        kernel.last_results = res
    return out
